# revision 11
# baseline (speedup 1.0000x reference)
"""DeepSeek-style MLA transformer block on 8 Trainium2 NeuronCores.

Strategy (feature-major activations on device; weights host-pre-transposed so
every matmul chains without on-device transposes):

  Stage A (token-sharded, 256 tok/core): attn_norm -> wq_a -> q_norm and
    wkv_a -> kv_norm + rope(k_pe); AllGather the small latents.
  Stage B (head-sharded, 2 heads/core): wq_b (+rope q), expand k_nope/v from
    the gathered kv latent, causal attention, AllGather y.
  Stage C (feature-sharded): wo on a 256-row output shard + residual,
    ffn-norm via AllReduce of sum-of-square partials, AllGather h2 (split in
    two token halves), fused SwiGLU MLP (TP over the 8192 inter dim, g kept
    in SBUF), ReduceScatter (x2 halves) + residual -> per-core [256, 2048]
    output shard; host stacks and transposes back.

All rmsnorm weights are folded into the following weight matrix on the host
(mathematically exact); softmax scale (-96) is folded into q at the wq_b
eviction; rmsnorm reduce+broadcast is one all-ones 128x128 matmul.

Matmuls run in float32r (1 cycle/row vs fp32's 4 when the moving dim is
>=256; measured rel err 1.5e-4 per matmul, tf32-class) gated by env flags
R_MLP/R_WO/R_V/R_ATTN (all default on).  The softmax itself and all psum
accumulation stay fp32.
"""

import os
import sys

sys.path.insert(0, "/opt/trn_rl_repo")

from contextlib import ExitStack

import numpy as np

import concourse.bacc as bacc
import concourse.bass as bass
import concourse.mybir as mybir
import concourse.tile as tile
from concourse.bass_utils import run_bass_kernel_spmd
from concourse.masks import make_identity

F32 = mybir.dt.float32
F32R = mybir.dt.float32r
AX = mybir.AxisListType.X
ADD = mybir.AluOpType.add
SUB = mybir.AluOpType.subtract
MUL = mybir.AluOpType.mult
AF = mybir.ActivationFunctionType

NCORES = 8
B, T, D = 2, 1024, 2048
H = 16
NOPE, ROPE = 128, 64
QKD = NOPE + ROPE  # 192
QLORA, KVLORA = 1536, 512
VHD = 128
INTER = 8192
EPS = 1e-6
SOFTSCALE = float(QKD) * -0.5  # -96.0

N_TOK = B * T  # 2048
S = N_TOK // NCORES  # 256 tokens per core (stage A shard)
HPC = H // NCORES  # 2 heads per core
DO = D // NCORES  # 256 output-feature rows per core
ISH = INTER // NCORES  # 1024 intermediate rows per core
AGQ = QLORA
AGKV = KVLORA + ROPE  # 576


def _cfg(name, default="1"):
    return os.environ.get(name, default) == "1"


_CACHED_NC = None
LAST_RESULTS = None  # test.py reads these
LAST_IN_MAPS = None


def _rms_inv(nc, ones128, eps_ap, psum_pool, work_pool, chunks, dim, n, tag):
    """chunks: list of APs [128, n] covering `dim` feature rows (feature-major).
    Returns an SBUF tile [128, n] whose every row is 1/sqrt(mean_sq + eps)."""
    ss = psum_pool.tile([128, n], F32, tag="rms_ss", name=f"{tag}_ss")
    nchunks = len(chunks)
    for i, xc in enumerate(chunks):
        xx = work_pool.tile([128, n], F32, tag="rms_xx", name=f"{tag}_xx")
        nc.scalar.square(xx[:], xc)
        nc.tensor.matmul(
            ss[:], ones128[:], xx[:], start=(i == 0), stop=(i == nchunks - 1)
        )
    std = work_pool.tile([128, n], F32, tag="rms_std", name=f"{tag}_std")
    nc.scalar.activation(std[:], ss[:], AF.Sqrt, bias=eps_ap, scale=1.0 / dim)
    inv = work_pool.tile([128, n], F32, tag=f"{tag}_inv", name=f"{tag}_inv")
    nc.vector.reciprocal(inv[:], std[:])
    return inv


def _f32v(ap):
    """View a (possibly float32r) AP as plain float32 for DVE/ACT reads."""
    return ap.bitcast(F32) if ap.dtype == F32R else ap


def _rope(nc, pool, out64, in64, cos, sin, n, tag):
    """out64/in64: APs [64, n]; rows 0:32 = even lanes, 32:64 = odd lanes.
    cos/sin: APs [32, n] at partition base 0.  in64 may sit at any 32-aligned
    base; walrus requires equal input bases for 2-input SBUF ops, so stage the
    halves through base-0 copies first (single-input ops may shift bases)."""
    in64 = _f32v(in64)
    qe = pool.tile([32, n], F32, tag="rope_qe", name=f"{tag}_qe")
    qo = pool.tile([32, n], F32, tag="rope_qo", name=f"{tag}_qo")
    nc.scalar.copy(qe[:], in64[0:32, :])
    nc.scalar.copy(qo[:], in64[32:64, :])
    t0 = pool.tile([32, n], F32, tag="rope_t0", name=f"{tag}_t0")
    t1 = pool.tile([32, n], F32, tag="rope_t1", name=f"{tag}_t1")
    nc.vector.tensor_tensor(t0[:], qe[:], cos, MUL)
    nc.vector.tensor_tensor(t1[:], qo[:], sin, MUL)
    nc.vector.tensor_tensor(out64[0:32, :], t0[:], t1[:], SUB)
    t2 = pool.tile([32, n], F32, tag="rope_t2", name=f"{tag}_t2")
    t3 = pool.tile([32, n], F32, tag="rope_t3", name=f"{tag}_t3")
    nc.vector.tensor_tensor(t2[:], qe[:], sin, MUL)
    nc.vector.tensor_tensor(t3[:], qo[:], cos, MUL)
    nc.vector.tensor_tensor(out64[32:64, :], t2[:], t3[:], ADD)


def _build():
    nc = bacc.Bacc("TRN2", target_bir_lowering=False, debug=False, num_devices=NCORES)
    RG = [list(range(NCORES))]
    r_mlp = _cfg("R_MLP")
    r_wo = _cfg("R_WO")
    r_attn = _cfg("R_ATTN")
    r_v = _cfg("R_V") or r_attn
    DT_MLP = F32R if r_mlp else F32
    DT_WO = F32R if r_wo else F32
    DT_V = F32R if r_v else F32
    DT_AT = F32R if r_attn else F32

    def bc(ap, dt):
        return ap.bitcast(dt) if dt == F32R else ap

    # ---- kernel parameters (per-core data supplied via in_maps) ----
    p_xs = nc.declare_dram_parameter("xs", [D, S], F32, isOutput=False)
    p_xf = nc.declare_dram_parameter("xf", [DO, N_TOK], F32, isOutput=False)
    p_mask = nc.declare_dram_parameter("mask", [T, T], F32, isOutput=False)
    p_cosq = nc.declare_dram_parameter("cosq", [32, T], F32, isOutput=False)
    p_sinq = nc.declare_dram_parameter("sinq", [32, T], F32, isOutput=False)
    p_cosk = nc.declare_dram_parameter("cosk", [32, S], F32, isOutput=False)
    p_sink = nc.declare_dram_parameter("sink", [32, S], F32, isOutput=False)
    p_wqa = nc.declare_dram_parameter("wqaT", [D, QLORA], F32, isOutput=False)
    p_wqb = nc.declare_dram_parameter("wqbT", [QLORA, HPC * QKD], F32, isOutput=False)
    p_wkva = nc.declare_dram_parameter("wkvaT", [D, AGKV], F32, isOutput=False)
    p_wkb = nc.declare_dram_parameter("wkbT", [KVLORA, HPC * NOPE], F32, isOutput=False)
    p_wv = nc.declare_dram_parameter("wvT", [KVLORA, HPC * VHD], F32, isOutput=False)
    p_wo = nc.declare_dram_parameter("woT", [H * VHD, DO], F32, isOutput=False)
    p_w13 = nc.declare_dram_parameter("w13T", [D, 2 * ISH], F32, isOutput=False)
    p_w2 = nc.declare_dram_parameter("w2T", [ISH, D], F32, isOutput=False)
    p_out = nc.declare_dram_parameter("out", [DO, N_TOK], F32, isOutput=True)

    with tile.TileContext(
        nc, trace_sim=bool(os.environ.get("TILE_TRACE_SIM"))
    ) as tc, ExitStack() as root:
        dram = root.enter_context(tc.tile_pool(name="dram", bufs=1, space="DRAM"))
        agq_in = dram.tile([AGQ, S], F32, name="agq_in")
        agq_out = dram.tile([NCORES * AGQ, S], F32, addr_space="Shared", name="agq_out")
        agkv_in = dram.tile([AGKV, S], F32, name="agkv_in")
        agkv_out = dram.tile(
            [NCORES * AGKV, S], F32, addr_space="Shared", name="agkv_out"
        )
        y_in = dram.tile([HPC * VHD, N_TOK], F32, name="y_in")
        y_out = dram.tile([H * VHD, N_TOK], F32, addr_space="Shared", name="y_out")
        ar_in = dram.tile([128, N_TOK], F32, name="ar_in")
        ar_out = dram.tile([128, N_TOK], F32, addr_space="Shared", name="ar_out")
        h2_in = [dram.tile([DO, T], F32, name=f"h2_in{h}") for h in range(2)]
        h2f = [
            dram.tile([D, T], F32, addr_space="Shared", name=f"h2f{h}")
            for h in range(2)
        ]
        rs_in = [dram.tile([D, T], F32, name=f"rs_in{h}") for h in range(2)]
        rs_out = [dram.tile([DO, T], F32, name=f"rs_out{h}") for h in range(2)]

        const = root.enter_context(tc.tile_pool(name="const", bufs=1))
        ones128 = const.tile([128, 128], F32, name="ones128")
        nc.vector.memset(ones128[:], 1.0)
        eps_sb = const.tile([128, 1], F32, name="eps_sb")
        nc.vector.memset(eps_sb[:], EPS)
        ident = const.tile([128, 128], F32, name="ident")
        make_identity(nc, ident[:])
        cosq = const.tile([32, T], F32, name="cosq")
        sinq = const.tile([32, T], F32, name="sinq")
        nc.sync.dma_start(cosq[:], p_cosq[:, :])
        nc.sync.dma_start(sinq[:], p_sinq[:, :])

        # =========================== Stage A ===========================
        with ExitStack() as sa:
            a_res = sa.enter_context(tc.tile_pool(name="a_res", bufs=1))
            a_wk = sa.enter_context(tc.tile_pool(name="a_wk", bufs=3))
            a_w = sa.enter_context(tc.tile_pool(name="a_w", bufs=4))
            a_ps = sa.enter_context(tc.tile_pool(name="a_ps", bufs=2, space="PSUM"))

            x_sb = a_res.tile([128, D // 128, S], F32, name="x_sb")
            nc.sync.dma_start(x_sb[:], p_xs.rearrange("(ko p) n -> p ko n", p=128))
            inv_a = _rms_inv(
                nc, ones128, eps_sb[:], a_ps, a_wk,
                [x_sb[:, k, :] for k in range(D // 128)], D, S, "rmsa",
            )
            xh = a_res.tile([128, D // 128, S], DT_AT, name="xh")
            for k in range(D // 128):
                nc.vector.tensor_tensor(xh[:, k, :], x_sb[:, k, :], inv_a[:], MUL)

            # q_lat = wq_a' @ xh   -> [1536, S] feature-major
            ql = a_res.tile([128, QLORA // 128, S], F32, name="ql")
            for m in range(QLORA // 128):
                ps = a_ps.tile([128, S], F32, tag="a_mm", name="ql_ps")
                for k in range(D // 128):
                    wt = a_w.tile([128, 128], DT_AT, tag="wqa_t", name="wqa_t")
                    nc.sync.dma_start(
                        wt[:],
                        bc(p_wqa[128 * k : 128 * (k + 1), 128 * m : 128 * (m + 1)], DT_AT),
                    )
                    nc.tensor.matmul(
                        ps[:], wt[:], xh[:, k, :],
                        start=(k == 0), stop=(k == D // 128 - 1),
                    )
                nc.any.tensor_copy(ql[:, m, :], ps[:])
            inv_q = _rms_inv(
                nc, ones128, eps_sb[:], a_ps, a_wk,
                [ql[:, m, :] for m in range(QLORA // 128)], QLORA, S, "rmsq",
            )
            for m in range(QLORA // 128):
                qh = a_wk.tile([128, S], F32, tag="qh", name="qh")
                nc.vector.tensor_tensor(qh[:], ql[:, m, :], inv_q[:], MUL)
                nc.sync.dma_start(agq_in[128 * m : 128 * (m + 1), :], qh[:])

            # kv_all = wkv_a' @ xh -> [576, S]; rows 0:512 latent, 512:576 rope key
            kv = a_res.tile([128, 5, S], F32, name="kv")  # 4x128 + 1x64 (in row 4)
            mchunks = [(0, 128), (128, 128), (256, 128), (384, 128), (512, 64)]
            for mi, (moff, msz) in enumerate(mchunks):
                ps = a_ps.tile([128, S], F32, tag="a_mm", name="kv_ps")
                for k in range(D // 128):
                    wt = a_w.tile([128, 128], DT_AT, tag="wkva_t", name="wkva_t")
                    nc.sync.dma_start(
                        wt[:, :msz],
                        bc(p_wkva[128 * k : 128 * (k + 1), moff : moff + msz], DT_AT),
                    )
                    nc.tensor.matmul(
                        ps[:msz, :], wt[:, :msz], xh[:, k, :],
                        start=(k == 0), stop=(k == D // 128 - 1),
                    )
                nc.any.tensor_copy(kv[:msz, mi, :], ps[:msz, :])
            inv_kv = _rms_inv(
                nc, ones128, eps_sb[:], a_ps, a_wk,
                [kv[:, m, :] for m in range(4)], KVLORA, S, "rmskv",
            )
            for m in range(4):
                ln = a_wk.tile([128, S], F32, tag="latn", name="latn")
                nc.vector.tensor_tensor(ln[:], kv[:, m, :], inv_kv[:], MUL)
                nc.sync.dma_start(agkv_in[128 * m : 128 * (m + 1), :], ln[:])
            # rope the decoupled key (shared across heads), feature-permuted on host
            cosk = a_res.tile([32, S], F32, name="cosk")
            sink = a_res.tile([32, S], F32, name="sink")
            nc.sync.dma_start(cosk[:], p_cosk[:, :])
            nc.sync.dma_start(sink[:], p_sink[:, :])
            kpe_r = a_wk.tile([64, S], F32, tag="kpe_r", name="kpe_r")
            _rope(nc, a_wk, kpe_r[:], kv[0:64, 4, :], cosk[:], sink[:], S, "ropek")
            nc.sync.dma_start(agkv_in[KVLORA : KVLORA + ROPE, :], kpe_r[:])

        nc.gpsimd.collective_compute(
            "AllGather", mybir.AluOpType.bypass, replica_groups=RG,
            ins=[agkv_in[:].opt()], outs=[agkv_out[:].opt()],
        )
        nc.gpsimd.collective_compute(
            "AllGather", mybir.AluOpType.bypass, replica_groups=RG,
            ins=[agq_in[:].opt()], outs=[agq_out[:].opt()],
        )

        # =========================== Stage B ===========================
        res1 = root.enter_context(tc.tile_pool(name="res1", bufs=1))
        res1_sb = res1.tile([128, DO // 128, N_TOK], F32, name="res1_sb")

        with ExitStack() as sb:
            b_res = sb.enter_context(tc.tile_pool(name="b_res", bufs=1))
            b_wk = sb.enter_context(tc.tile_pool(name="b_wk", bufs=2))

            k_sb = b_res.tile([128, HPC, N_TOK], DT_AT, name="k_sb")
            kpe_sb = b_res.tile([128, N_TOK], DT_AT, name="kpe_sb")
            v_sb = b_res.tile([128, N_TOK // 128, HPC * VHD], F32, name="v_sb")
            q_sb = b_res.tile([128, 3, N_TOK], DT_AT, name="q_sb")
            qpe_sb = b_res.tile([128, N_TOK], DT_AT, name="qpe_sb")
            y_sb = b_res.tile([128, HPC, N_TOK], F32, name="y_sb")

            # ---- expand k_nope and v (token-major) for the 2 local heads ----
            with ExitStack() as s1:
                b_w1 = s1.enter_context(tc.tile_pool(name="b_w1", bufs=1))
                b_rhs1 = s1.enter_context(tc.tile_pool(name="b_rhs1", bufs=8))
                b1_ps = s1.enter_context(
                    tc.tile_pool(name="b1_ps", bufs=2, space="PSUM")
                )
                wkb_sb = b_w1.tile([128, 4, HPC * NOPE], DT_V, name="wkb_sb")
                nc.sync.dma_start(
                    wkb_sb[:], bc(p_wkb.rearrange("(ko p) m -> p ko m", p=128), DT_V)
                )
                wv_sb = b_w1.tile([128, 4, HPC * VHD], DT_V, name="wv_sb")
                nc.sync.dma_start(
                    wv_sb[:], bc(p_wv.rearrange("(ko p) m -> p ko m", p=128), DT_V)
                )
                for blk in range(NCORES):
                    base = AGKV * blk
                    lat = [
                        b_rhs1.tile([128, S], DT_V, tag="lat", name="lat")
                        for _ in range(4)
                    ]
                    for k in range(4):
                        nc.sync.dma_start(
                            lat[k][:],
                            bc(agkv_out[base + 128 * k : base + 128 * (k + 1), :], DT_V),
                        )
                    for m in range(HPC):
                        psk = b1_ps.tile([128, S], F32, tag="psk", name="psk")
                        for k in range(4):
                            nc.tensor.matmul(
                                psk[:], wkb_sb[:, k, 128 * m : 128 * (m + 1)],
                                lat[k][:], start=(k == 0), stop=(k == 3),
                            )
                        nc.any.tensor_copy(k_sb[:, m, S * blk : S * (blk + 1)], psk[:])
                    for th in range(S // 128):
                        psv = b1_ps.tile([128, HPC * VHD], F32, tag="psv", name="psv")
                        for k in range(4):
                            nc.tensor.matmul(
                                psv[:], lat[k][:, 128 * th : 128 * (th + 1)],
                                wv_sb[:, k, :], start=(k == 0), stop=(k == 3),
                            )
                        nc.any.tensor_copy(v_sb[:, (S // 128) * blk + th, :], psv[:])
                    nc.sync.dma_start(
                        kpe_sb[0:64, S * blk : S * (blk + 1)],
                        bc(agkv_out[base + KVLORA : base + KVLORA + ROPE, :], DT_AT),
                    )
                    nc.sync.dma_start(
                        kpe_sb[64:128, S * blk : S * (blk + 1)],
                        bc(agkv_out[base + KVLORA : base + KVLORA + ROPE, :], DT_AT),
                    )

            # ---- q for the 2 local heads (fold SOFTSCALE here) ----
            with ExitStack() as s2:
                b_w2 = s2.enter_context(tc.tile_pool(name="b_w2", bufs=1))
                b_rhs2 = s2.enter_context(tc.tile_pool(name="b_rhs2", bufs=8))
                b2_ps = s2.enter_context(
                    tc.tile_pool(name="b2_ps", bufs=2, space="PSUM")
                )
                wqb_sb = b_w2.tile([128, QLORA // 128, HPC * QKD], DT_AT, name="wqb_sb")
                nc.sync.dma_start(
                    wqb_sb[:], bc(p_wqb.rearrange("(ko p) m -> p ko m", p=128), DT_AT)
                )
                for blk in range(NCORES):
                    base = AGQ * blk
                    qrhs = [
                        b_rhs2.tile([128, S], DT_AT, tag="qrhs", name="qrhs")
                        for _ in range(QLORA // 128)
                    ]
                    for k in range(QLORA // 128):
                        nc.sync.dma_start(
                            qrhs[k][:],
                            bc(agq_out[base + 128 * k : base + 128 * (k + 1), :], DT_AT),
                        )
                    psq = [
                        b2_ps.tile([128, S], F32, tag=f"psq{m}", name=f"psq{m}")
                        for m in range(3)
                    ]
                    for k in range(QLORA // 128):
                        for m in range(3):
                            nc.tensor.matmul(
                                psq[m][:], wqb_sb[:, k, 128 * m : 128 * (m + 1)],
                                qrhs[k][:],
                                start=(k == 0), stop=(k == QLORA // 128 - 1),
                            )
                    for m in range(3):
                        nc.scalar.mul(
                            q_sb[:, m, S * blk : S * (blk + 1)], psq[m][:], SOFTSCALE
                        )

            # rope q_pe: q_sb chunk 2 = [h0_e, h0_o, h1_e, h1_o] x32 rows
            with tc.tile_pool(name="rope_wk", bufs=1) as rp:
                for b in range(B):
                    for h in range(HPC):
                        _rope(
                            nc, rp,
                            qpe_sb[64 * h : 64 * (h + 1), T * b : T * (b + 1)],
                            q_sb[64 * h : 64 * (h + 1), 2, T * b : T * (b + 1)],
                            cosq[:, :], sinq[:, :], T, f"ropeq{b}{h}",
                        )

            # ---- attention ----
            with ExitStack() as s3:
                b3_ps = s3.enter_context(
                    tc.tile_pool(name="b3_ps", bufs=2, space="PSUM")
                )
                for sc in range(T // 128):
                    W = 512 * (sc // 4 + 1)
                    mask_sb = b_wk.tile([128, T], F32, tag="mask_sb", name="mask_sb")
                    nc.sync.dma_start(
                        mask_sb[:, :W], p_mask[128 * sc : 128 * (sc + 1), :W]
                    )
                    for b in range(B):
                        s0 = T * b + 128 * sc
                        for h in range(HPC):
                            scs = b_wk.tile([128, T], F32, tag="scs", name="scs")
                            for tcx in range(W // 512):
                                t0 = T * b + 512 * tcx
                                ps = b3_ps.tile([128, 512], F32, tag="ps_qk", name="ps_qk")
                                nc.tensor.matmul(
                                    ps[:],
                                    q_sb[:, h, s0 : s0 + 128],
                                    k_sb[:, h, t0 : t0 + 512],
                                    start=True, stop=False,
                                )
                                nc.tensor.matmul(
                                    ps[:],
                                    qpe_sb[64 * h : 64 * (h + 1), s0 : s0 + 128],
                                    kpe_sb[64 * h : 64 * (h + 1), t0 : t0 + 512],
                                    start=False, stop=True,
                                )
                                nc.vector.tensor_tensor(
                                    scs[:, 512 * tcx : 512 * (tcx + 1)], ps[:],
                                    mask_sb[:, 512 * tcx : 512 * (tcx + 1)], ADD,
                                )
                            nmax = b_wk.tile([128, 1], F32, tag="nmax", name="nmax")
                            nc.vector.reduce_max(
                                nmax[:], scs[:, :W], axis=AX, negate=True
                            )
                            p_sb = b_wk.tile([128, T], F32, tag="p_sb", name="p_sb")
                            zsum = b_wk.tile([128, 1], F32, tag="zsum", name="zsum")
                            nc.scalar.activation(
                                p_sb[:, :W], scs[:, :W], AF.Exp,
                                bias=nmax[:], accum_out=zsum[:],
                            )
                            invz = b_wk.tile([128, 1], F32, tag="invz", name="invz")
                            nc.vector.reciprocal(invz[:], zsum[:])
                            nc.vector.tensor_scalar_mul(
                                p_sb[:, :W], p_sb[:, :W], invz[:]
                            )
                            ptr = b_wk.tile([128, T], F32, tag="ptr", name="ptr")
                            for tj in range(W // 128):
                                pst = b3_ps.tile([128, 128], F32, tag="pst", name="pst")
                                nc.tensor.transpose(
                                    pst[:], p_sb[:, 128 * tj : 128 * (tj + 1)], ident[:]
                                )
                                nc.any.tensor_copy(
                                    ptr[:, 128 * tj : 128 * (tj + 1)], pst[:]
                                )
                            psy = b3_ps.tile([128, 128], F32, tag="psy", name="psy")
                            ntj = W // 128
                            for tj in range(ntj):
                                nc.tensor.matmul(
                                    psy[:],
                                    v_sb[:, (T // 128) * b + tj, VHD * h : VHD * (h + 1)],
                                    ptr[:, 128 * tj : 128 * (tj + 1)],
                                    start=(tj == 0), stop=(tj == ntj - 1),
                                )
                            nc.any.tensor_copy(y_sb[:, h, s0 : s0 + 128], psy[:])

            for h in range(HPC):
                nc.sync.dma_start(y_in[VHD * h : VHD * (h + 1), :], y_sb[:, h, :])

        nc.gpsimd.collective_compute(
            "AllGather", mybir.AluOpType.bypass, replica_groups=RG,
            ins=[y_in[:].opt()], outs=[y_out[:].opt()],
        )

        # ================== Stage C: wo + norm + MLP ==================
        with ExitStack() as sc_stack:
            c_res = sc_stack.enter_context(tc.tile_pool(name="c_res", bufs=1))
            c_wk = sc_stack.enter_context(tc.tile_pool(name="c_wk", bufs=3))
            c_ps = sc_stack.enter_context(tc.tile_pool(name="c_ps", bufs=2, space="PSUM"))

            wo_sb = c_res.tile([128, H * VHD // 128, DO], DT_WO, name="wo_sb")
            nc.sync.dma_start(
                wo_sb[:], bc(p_wo.rearrange("(ko p) m -> p ko m", p=128), DT_WO)
            )
            for n in range(N_TOK // 512):
                pso = [
                    c_ps.tile([128, 512], F32, tag=f"pso{m}", name=f"pso{m}")
                    for m in range(DO // 128)
                ]
                for k in range(H * VHD // 128):
                    yr = c_wk.tile([128, 512], DT_WO, tag="yr", name="yr")
                    nc.sync.dma_start(
                        yr[:],
                        bc(y_out[128 * k : 128 * (k + 1), 512 * n : 512 * (n + 1)], DT_WO),
                    )
                    for m in range(DO // 128):
                        nc.tensor.matmul(
                            pso[m][:], wo_sb[:, k, 128 * m : 128 * (m + 1)], yr[:],
                            start=(k == 0), stop=(k == H * VHD // 128 - 1),
                        )
                for m in range(DO // 128):
                    xt = c_wk.tile([128, 512], F32, tag="xt", name="xt")
                    nc.sync.dma_start(
                        xt[:], p_xf[128 * m : 128 * (m + 1), 512 * n : 512 * (n + 1)]
                    )
                    nc.vector.tensor_tensor(
                        res1_sb[:, m, 512 * n : 512 * (n + 1)], pso[m][:], xt[:], ADD
                    )

            # ffn-norm partial sum of squares (rows replicated via all-ones mm)
            ssp = c_res.tile([128, N_TOK], F32, name="ssp")
            for n in range(N_TOK // 512):
                pss = c_ps.tile([128, 512], F32, tag="pss", name="pss")
                for m in range(DO // 128):
                    xx = c_wk.tile([128, 512], F32, tag="ffn_xx", name="ffn_xx")
                    nc.scalar.square(xx[:], res1_sb[:, m, 512 * n : 512 * (n + 1)])
                    nc.tensor.matmul(
                        pss[:], ones128[:], xx[:],
                        start=(m == 0), stop=(m == DO // 128 - 1),
                    )
                nc.any.tensor_copy(ssp[:, 512 * n : 512 * (n + 1)], pss[:])
            nc.sync.dma_start(ar_in[:], ssp[:])
            nc.gpsimd.collective_compute(
                "AllReduce", mybir.AluOpType.add, replica_groups=RG,
                ins=[ar_in[:].opt()], outs=[ar_out[:].opt()],
            )
            ssf = c_res.tile([128, N_TOK], F32, name="ssf")
            nc.sync.dma_start(ssf[:], ar_out[:])
            stdf = c_res.tile([128, N_TOK], F32, name="stdf")
            nc.scalar.activation(stdf[:], ssf[:], AF.Sqrt, bias=eps_sb[:], scale=1.0 / D)
            invf = c_res.tile([128, N_TOK], F32, name="invf")
            nc.vector.reciprocal(invf[:], stdf[:])
            for half in range(2):
                for m in range(DO // 128):
                    h2t = c_wk.tile([128, T], F32, tag="h2t", name="h2t")
                    nc.vector.tensor_tensor(
                        h2t[:], res1_sb[:, m, T * half : T * (half + 1)],
                        invf[:, T * half : T * (half + 1)], MUL,
                    )
                    nc.sync.dma_start(h2_in[half][128 * m : 128 * (m + 1), :], h2t[:])
                nc.gpsimd.collective_compute(
                    "AllGather", mybir.AluOpType.bypass, replica_groups=RG,
                    ins=[h2_in[half][:].opt()], outs=[h2f[half][:].opt()],
                )

        # ---- MLP (TP over INTER), one pass per token half ----
        for half in range(2):
            with ExitStack() as sm:
                m_res = sm.enter_context(tc.tile_pool(name=f"m_res{half}", bufs=1))
                m_w = sm.enter_context(tc.tile_pool(name=f"m_w{half}", bufs=2))
                m_wk = sm.enter_context(tc.tile_pool(name=f"m_wk{half}", bufs=3))
                m_ps = sm.enter_context(
                    tc.tile_pool(name=f"m_ps{half}", bufs=2, space="PSUM")
                )
                hc = m_res.tile([128, D // 128, T], DT_MLP, name=f"hc{half}")
                nc.sync.dma_start(
                    hc[:],
                    bc(h2f[half][:].rearrange("(ko p) n -> p ko n", p=128), DT_MLP),
                )
                g_sb = m_res.tile([128, ISH // 128, T], DT_MLP, name=f"g{half}")
                for j in range(ISH // 128):
                    wj = m_w.tile([128, D // 128, 256], DT_MLP, tag="wj", name="wj")
                    nc.sync.dma_start(
                        wj[:],
                        bc(
                            p_w13[:, 256 * j : 256 * (j + 1)].rearrange(
                                "(ko p) m -> p ko m", p=128
                            ),
                            DT_MLP,
                        ),
                    )
                    for ns in range(T // 512):
                        psa = m_ps.tile([128, 512], F32, tag="psa", name="psa")
                        psb = m_ps.tile([128, 512], F32, tag="psb", name="psb")
                        for k in range(D // 128):
                            nc.tensor.matmul(
                                psa[:], wj[:, k, 0:128],
                                hc[:, k, 512 * ns : 512 * (ns + 1)],
                                start=(k == 0), stop=(k == D // 128 - 1),
                            )
                            nc.tensor.matmul(
                                psb[:], wj[:, k, 128:256],
                                hc[:, k, 512 * ns : 512 * (ns + 1)],
                                start=(k == 0), stop=(k == D // 128 - 1),
                            )
                        tsi = m_wk.tile([128, 512], F32, tag="tsi", name="tsi")
                        nc.scalar.activation(tsi[:], psa[:], AF.Silu)
                        nc.vector.tensor_tensor(
                            g_sb[:, j, 512 * ns : 512 * (ns + 1)], tsi[:], psb[:], MUL
                        )
                for m in range(D // 128):
                    w2m = m_w.tile([128, ISH // 128, 128], DT_MLP, tag="w2m", name="w2m")
                    nc.sync.dma_start(
                        w2m[:],
                        bc(
                            p_w2[:, 128 * m : 128 * (m + 1)].rearrange(
                                "(ko p) m2 -> p ko m2", p=128
                            ),
                            DT_MLP,
                        ),
                    )
                    for ns in range(T // 512):
                        ps2 = m_ps.tile([128, 512], F32, tag="ps2", name="ps2")
                        for k in range(ISH // 128):
                            nc.tensor.matmul(
                                ps2[:], w2m[:, k, :],
                                g_sb[:, k, 512 * ns : 512 * (ns + 1)],
                                start=(k == 0), stop=(k == ISH // 128 - 1),
                            )
                        o_sb = m_wk.tile([128, 512], F32, tag="o_sb", name="o_sb")
                        nc.any.tensor_copy(o_sb[:], ps2[:])
                        nc.sync.dma_start(
                            rs_in[half][
                                128 * m : 128 * (m + 1), 512 * ns : 512 * (ns + 1)
                            ],
                            o_sb[:],
                        )
            nc.gpsimd.collective_compute(
                "ReduceScatter", mybir.AluOpType.add, replica_groups=RG,
                ins=[rs_in[half][:].opt()], outs=[rs_out[half][:].opt()],
            )

        # ---- final residual add + output ----
        with ExitStack() as sf:
            f_wk = sf.enter_context(tc.tile_pool(name="f_wk", bufs=2))
            for half in range(2):
                for m in range(DO // 128):
                    rt = f_wk.tile([128, T], F32, tag="rt", name="rt")
                    nc.sync.dma_start(rt[:], rs_out[half][128 * m : 128 * (m + 1), :])
                    ot = f_wk.tile([128, T], F32, tag="ot", name="ot")
                    nc.vector.tensor_tensor(
                        ot[:], rt[:], res1_sb[:, m, T * half : T * (half + 1)], ADD
                    )
                    nc.sync.dma_start(
                        p_out[128 * m : 128 * (m + 1), T * half : T * (half + 1)], ot[:]
                    )

    nc.compile()
    return nc


def _rope_perm(n):
    """Permutation putting even lanes first then odd lanes, for an n-row rope
    block (n even): [0,2,4,...,n-2, 1,3,5,...,n-1]."""
    return np.concatenate([np.arange(0, n, 2), np.arange(1, n, 2)])


def kernel(**inputs):
    global _CACHED_NC, LAST_RESULTS, LAST_IN_MAPS
    f32 = lambda a: np.ascontiguousarray(np.asarray(a), dtype=np.float32)

    x = f32(inputs["x"]).reshape(N_TOK, D)
    mask = f32(inputs["mask"])
    cos = f32(inputs["freqs_cos"])  # [T, 32]
    sin = f32(inputs["freqs_sin"])
    attn_nw = f32(inputs["attn_norm_w"])
    wq_a = f32(inputs["wq_a"]) * attn_nw[None, :]
    q_nw = f32(inputs["q_norm_w"])
    wq_b = f32(inputs["wq_b"]) * q_nw[None, :]
    wkv_a = f32(inputs["wkv_a"]) * attn_nw[None, :]
    kv_nw = f32(inputs["kv_norm_w"])
    wkv_b = f32(inputs["wkv_b"]) * kv_nw[None, :]
    wo = f32(inputs["wo"])
    ffn_nw = f32(inputs["ffn_norm_w"])
    w1 = f32(inputs["w1"]) * ffn_nw[None, :]
    w3 = f32(inputs["w3"]) * ffn_nw[None, :]
    w2 = f32(inputs["w2"])

    xT = np.ascontiguousarray(x.T)  # [D, N_TOK] feature-major
    cosT = np.ascontiguousarray(cos.T)  # [32, T]
    sinT = np.ascontiguousarray(sin.T)

    # wkv_a rows: keep 0:512 (latent); permute rope rows 512:576 to even|odd
    pk = _rope_perm(ROPE)
    wkva_p = wkv_a.copy()
    wkva_p[KVLORA:] = wkv_a[KVLORA:][pk]
    wkvaT = np.ascontiguousarray(wkva_p.T)  # [D, 576]

    wqaT = np.ascontiguousarray(wq_a.T)  # [D, 1536]

    in_maps = []
    for c in range(NCORES):
        heads = [HPC * c + j for j in range(HPC)]
        # wq_b rows per head h: h*QKD .. h*QKD+192 (128 nope + 64 rope)
        # target col order: [h0_nope(128), h1_nope(128), h0_rope_eo(64), h1_rope_eo(64)]
        cols = []
        for h in heads:
            cols.append(wq_b[h * QKD : h * QKD + NOPE])
        for h in heads:
            cols.append(wq_b[h * QKD + NOPE : (h + 1) * QKD][pk])
        wqbT = np.ascontiguousarray(np.concatenate(cols, axis=0).T)  # [1536, 384]

        # wkv_b rows per head h: h*(NOPE+VHD) + [0:128]=k_nope, [128:256]=v
        kw = np.concatenate(
            [wkv_b[h * (NOPE + VHD) : h * (NOPE + VHD) + NOPE] for h in heads], axis=0
        )
        vw = np.concatenate(
            [wkv_b[h * (NOPE + VHD) + NOPE : (h + 1) * (NOPE + VHD)] for h in heads],
            axis=0,
        )
        wkbT = np.ascontiguousarray(kw.T)  # [512, 256]
        wvT = np.ascontiguousarray(vw.T)  # [512, 256]

        woT = np.ascontiguousarray(wo[DO * c : DO * (c + 1)].T)  # [2048, 256]

        w1s = w1[ISH * c : ISH * (c + 1)]  # [1024, 2048]
        w3s = w3[ISH * c : ISH * (c + 1)]
        w13T = np.empty((D, 2 * ISH), np.float32)
        for j in range(ISH // 128):
            w13T[:, 256 * j : 256 * j + 128] = w1s[128 * j : 128 * (j + 1)].T
            w13T[:, 256 * j + 128 : 256 * (j + 1)] = w3s[128 * j : 128 * (j + 1)].T
        w2T = np.ascontiguousarray(w2[:, ISH * c : ISH * (c + 1)].T)  # [1024, 2048]

        tpos = (S * c) % T  # position within batch of this token shard
        in_maps.append(
            {
                "xs": np.ascontiguousarray(xT[:, S * c : S * (c + 1)]),
                "xf": np.ascontiguousarray(xT[DO * c : DO * (c + 1), :]),
                "mask": mask,
                "cosq": cosT,
                "sinq": sinT,
                "cosk": np.ascontiguousarray(cosT[:, tpos : tpos + S]),
                "sink": np.ascontiguousarray(sinT[:, tpos : tpos + S]),
                "wqaT": wqaT,
                "wqbT": wqbT,
                "wkvaT": wkvaT,
                "wkbT": wkbT,
                "wvT": wvT,
                "woT": woT,
                "w13T": np.ascontiguousarray(w13T),
                "w2T": w2T,
            }
        )

    LAST_IN_MAPS = in_maps
    if _CACHED_NC is None:
        _CACHED_NC = _build()
    nc = _CACHED_NC

    trace = bool(os.environ.get("KERNEL_TRACE"))
    res = run_bass_kernel_spmd(
        nc, in_maps, core_ids=list(range(NCORES)), trace=trace
    )
    LAST_RESULTS = res

    outT = np.concatenate([res.results[c]["out"] for c in range(NCORES)], axis=0)
    return np.ascontiguousarray(outT.T).reshape(B, T, D).astype(np.float32)


# revision 13
# speedup vs baseline: 1.0495x; 1.0495x over previous
"""DeepSeek-style MLA transformer block on 8 Trainium2 NeuronCores.

Strategy (feature-major activations on device; weights host-pre-transposed so
every matmul chains without on-device transposes):

  Stage A (token-sharded, 256 tok/core): attn_norm -> wq_a -> q_norm and
    wkv_a -> kv_norm + rope(k_pe); AllGather the small latents.
  Stage B (head-sharded, 2 heads/core): wq_b (+rope q), expand k_nope/v from
    the gathered kv latent, causal attention, AllGather y.
  Stage C (feature-sharded): wo on a 256-row output shard + residual,
    ffn-norm via AllReduce of sum-of-square partials, AllGather h2 (split in
    two token halves), fused SwiGLU MLP (TP over the 8192 inter dim, g kept
    in SBUF), ReduceScatter (x2 halves) + residual -> per-core [256, 2048]
    output shard; host stacks and transposes back.

All rmsnorm weights are folded into the following weight matrix on the host
(mathematically exact); softmax scale (-96) is folded into q at the wq_b
eviction; rmsnorm reduce+broadcast is one all-ones 128x128 matmul.

Matmuls run in float32r (1 cycle/row vs fp32's 4 when the moving dim is
>=256; measured rel err 1.5e-4 per matmul, tf32-class) gated by env flags
R_MLP/R_WO/R_V/R_ATTN (all default on).  The softmax itself and all psum
accumulation stay fp32.
"""

import os
import sys

sys.path.insert(0, "/opt/trn_rl_repo")

from contextlib import ExitStack

import numpy as np

import concourse.bacc as bacc
import concourse.bass as bass
import concourse.mybir as mybir
import concourse.tile as tile
from concourse.bass_utils import run_bass_kernel_spmd
from concourse.masks import make_identity

F32 = mybir.dt.float32
F32R = mybir.dt.float32r
AX = mybir.AxisListType.X
ADD = mybir.AluOpType.add
SUB = mybir.AluOpType.subtract
MUL = mybir.AluOpType.mult
AF = mybir.ActivationFunctionType

NCORES = 8
B, T, D = 2, 1024, 2048
H = 16
NOPE, ROPE = 128, 64
QKD = NOPE + ROPE  # 192
QLORA, KVLORA = 1536, 512
VHD = 128
INTER = 8192
EPS = 1e-6
SOFTSCALE = float(QKD) * -0.5  # -96.0

N_TOK = B * T  # 2048
S = N_TOK // NCORES  # 256 tokens per core (stage A shard)
HPC = H // NCORES  # 2 heads per core
DO = D // NCORES  # 256 output-feature rows per core
ISH = INTER // NCORES  # 1024 intermediate rows per core
AGQ = QLORA
AGKV = KVLORA + ROPE  # 576


def _cfg(name, default="1"):
    return os.environ.get(name, default) == "1"


_CACHED_NC = None
LAST_RESULTS = None  # test.py reads these
LAST_IN_MAPS = None


def _rms_inv(nc, ones128, eps_ap, psum_pool, work_pool, chunks, dim, n, tag):
    """chunks: list of APs [128, n] covering `dim` feature rows (feature-major).
    Returns an SBUF tile [128, n] whose every row is 1/sqrt(mean_sq + eps)."""
    ss = psum_pool.tile([128, n], F32, tag="rms_ss", name=f"{tag}_ss")
    nchunks = len(chunks)
    for i, xc in enumerate(chunks):
        xx = work_pool.tile([128, n], F32, tag="rms_xx", name=f"{tag}_xx")
        nc.scalar.square(xx[:], xc)
        nc.tensor.matmul(
            ss[:], ones128[:], xx[:], start=(i == 0), stop=(i == nchunks - 1)
        )
    std = work_pool.tile([128, n], F32, tag="rms_std", name=f"{tag}_std")
    nc.scalar.activation(std[:], ss[:], AF.Sqrt, bias=eps_ap, scale=1.0 / dim)
    inv = work_pool.tile([128, n], F32, tag=f"{tag}_inv", name=f"{tag}_inv")
    nc.vector.reciprocal(inv[:], std[:])
    return inv


def _f32v(ap):
    """View a (possibly float32r) AP as plain float32 for DVE/ACT reads."""
    return ap.bitcast(F32) if ap.dtype == F32R else ap


def _rope(nc, pool, out64, in64, cos, sin, n, tag):
    """out64/in64: APs [64, n]; rows 0:32 = even lanes, 32:64 = odd lanes.
    cos/sin: APs [32, n] at partition base 0.  in64 may sit at any 32-aligned
    base; walrus requires equal input bases for 2-input SBUF ops, so stage the
    halves through base-0 copies first (single-input ops may shift bases)."""
    in64 = _f32v(in64)
    qe = pool.tile([32, n], F32, tag="rope_qe", name=f"{tag}_qe")
    qo = pool.tile([32, n], F32, tag="rope_qo", name=f"{tag}_qo")
    nc.scalar.copy(qe[:], in64[0:32, :])
    nc.scalar.copy(qo[:], in64[32:64, :])
    t0 = pool.tile([32, n], F32, tag="rope_t0", name=f"{tag}_t0")
    t1 = pool.tile([32, n], F32, tag="rope_t1", name=f"{tag}_t1")
    nc.vector.tensor_tensor(t0[:], qe[:], cos, MUL)
    nc.vector.tensor_tensor(t1[:], qo[:], sin, MUL)
    nc.vector.tensor_tensor(out64[0:32, :], t0[:], t1[:], SUB)
    t2 = pool.tile([32, n], F32, tag="rope_t2", name=f"{tag}_t2")
    t3 = pool.tile([32, n], F32, tag="rope_t3", name=f"{tag}_t3")
    nc.vector.tensor_tensor(t2[:], qe[:], sin, MUL)
    nc.vector.tensor_tensor(t3[:], qo[:], cos, MUL)
    nc.vector.tensor_tensor(out64[32:64, :], t2[:], t3[:], ADD)


def _build():
    nc = bacc.Bacc("TRN2", target_bir_lowering=False, debug=False, num_devices=NCORES)
    RG = [list(range(NCORES))]
    r_mlp = _cfg("R_MLP")
    r_wo = _cfg("R_WO")
    r_attn = _cfg("R_ATTN", "0")
    r_v = _cfg("R_V") or r_attn
    DT_MLP = F32R if r_mlp else F32
    DT_WO = F32R if r_wo else F32
    DT_V = F32R if r_v else F32
    DT_AT = F32R if r_attn else F32

    def bc(ap, dt):
        return ap.bitcast(dt) if dt == F32R else ap

    # ---- kernel parameters (per-core data supplied via in_maps) ----
    p_xs = nc.declare_dram_parameter("xs", [D, S], F32, isOutput=False)
    p_xf = nc.declare_dram_parameter("xf", [DO, N_TOK], F32, isOutput=False)
    p_mask = nc.declare_dram_parameter("mask", [T, T], F32, isOutput=False)
    p_cosq = nc.declare_dram_parameter("cosq", [32, T], F32, isOutput=False)
    p_sinq = nc.declare_dram_parameter("sinq", [32, T], F32, isOutput=False)
    p_cosk = nc.declare_dram_parameter("cosk", [32, S], F32, isOutput=False)
    p_sink = nc.declare_dram_parameter("sink", [32, S], F32, isOutput=False)
    p_wqa = nc.declare_dram_parameter("wqaT", [D, QLORA], F32, isOutput=False)
    p_wqb = nc.declare_dram_parameter("wqbT", [QLORA, HPC * QKD], F32, isOutput=False)
    p_wkva = nc.declare_dram_parameter("wkvaT", [D, AGKV], F32, isOutput=False)
    p_wkb = nc.declare_dram_parameter("wkbT", [KVLORA, HPC * NOPE], F32, isOutput=False)
    p_wv = nc.declare_dram_parameter("wvT", [KVLORA, HPC * VHD], F32, isOutput=False)
    p_wo = nc.declare_dram_parameter("woT", [H * VHD, DO], F32, isOutput=False)
    p_w13 = nc.declare_dram_parameter("w13T", [D, 2 * ISH], F32, isOutput=False)
    p_w2 = nc.declare_dram_parameter("w2T", [ISH, D], F32, isOutput=False)
    p_out = nc.declare_dram_parameter("out", [DO, N_TOK], F32, isOutput=True)

    with tile.TileContext(
        nc, trace_sim=bool(os.environ.get("TILE_TRACE_SIM"))
    ) as tc, ExitStack() as root:
        dram = root.enter_context(tc.tile_pool(name="dram", bufs=1, space="DRAM"))
        agq_in = dram.tile([AGQ, S], F32, name="agq_in")
        agq_out = dram.tile([NCORES * AGQ, S], F32, addr_space="Shared", name="agq_out")
        agkv_in = dram.tile([AGKV, S], F32, name="agkv_in")
        agkv_out = dram.tile(
            [NCORES * AGKV, S], F32, addr_space="Shared", name="agkv_out"
        )
        y_in = [dram.tile([VHD, N_TOK], F32, name=f"y_in{h}") for h in range(HPC)]
        y_out = [
            dram.tile([H * VHD // 2, N_TOK], F32, addr_space="Shared", name=f"y_out{h}")
            for h in range(HPC)
        ]
        ar_in = dram.tile([128, N_TOK], F32, name="ar_in")
        ar_out = dram.tile([128, N_TOK], F32, addr_space="Shared", name="ar_out")
        h2_in = [dram.tile([DO, T], F32, name=f"h2_in{h}") for h in range(2)]
        h2f = [
            dram.tile([D, T], F32, addr_space="Shared", name=f"h2f{h}")
            for h in range(2)
        ]
        rs_in = [dram.tile([D, T], F32, name=f"rs_in{h}") for h in range(2)]
        rs_out = [dram.tile([DO, T], F32, name=f"rs_out{h}") for h in range(2)]

        const = root.enter_context(tc.tile_pool(name="const", bufs=1))
        ones128 = const.tile([128, 128], F32, name="ones128")
        nc.vector.memset(ones128[:], 1.0)
        eps_sb = const.tile([128, 1], F32, name="eps_sb")
        nc.vector.memset(eps_sb[:], EPS)
        ident = const.tile([128, 128], F32, name="ident")
        make_identity(nc, ident[:])
        cosq = const.tile([32, T], F32, name="cosq")
        sinq = const.tile([32, T], F32, name="sinq")
        nc.sync.dma_start(cosq[:], p_cosq[:, :])
        nc.sync.dma_start(sinq[:], p_sinq[:, :])

        # =========================== Stage A ===========================
        with ExitStack() as sa:
            a_res = sa.enter_context(tc.tile_pool(name="a_res", bufs=1))
            a_wk = sa.enter_context(tc.tile_pool(name="a_wk", bufs=3))
            a_w = sa.enter_context(tc.tile_pool(name="a_w", bufs=4))
            a_ps = sa.enter_context(tc.tile_pool(name="a_ps", bufs=2, space="PSUM"))

            x_sb = a_res.tile([128, D // 128, S], F32, name="x_sb")
            nc.sync.dma_start(x_sb[:], p_xs.rearrange("(ko p) n -> p ko n", p=128))
            inv_a = _rms_inv(
                nc, ones128, eps_sb[:], a_ps, a_wk,
                [x_sb[:, k, :] for k in range(D // 128)], D, S, "rmsa",
            )
            xh = a_res.tile([128, D // 128, S], DT_AT, name="xh")
            for k in range(D // 128):
                nc.vector.tensor_tensor(xh[:, k, :], x_sb[:, k, :], inv_a[:], MUL)

            # kv_all = wkv_a' @ xh -> [576, S]; rows 0:512 latent, 512:576 rope key
            kv = a_res.tile([128, 5, S], F32, name="kv")  # 4x128 + 1x64 (in row 4)
            mchunks = [(0, 128), (128, 128), (256, 128), (384, 128), (512, 64)]
            for mi, (moff, msz) in enumerate(mchunks):
                ps = a_ps.tile([128, S], F32, tag="a_mm", name="kv_ps")
                for k in range(D // 128):
                    wt = a_w.tile([128, 128], DT_AT, tag="wkva_t", name="wkva_t")
                    nc.sync.dma_start(
                        wt[:, :msz],
                        bc(p_wkva[128 * k : 128 * (k + 1), moff : moff + msz], DT_AT),
                    )
                    nc.tensor.matmul(
                        ps[:msz, :], wt[:, :msz], xh[:, k, :],
                        start=(k == 0), stop=(k == D // 128 - 1),
                    )
                nc.any.tensor_copy(kv[:msz, mi, :], ps[:msz, :])
            inv_kv = _rms_inv(
                nc, ones128, eps_sb[:], a_ps, a_wk,
                [kv[:, m, :] for m in range(4)], KVLORA, S, "rmskv",
            )
            for m in range(4):
                ln = a_wk.tile([128, S], F32, tag="latn", name="latn")
                nc.vector.tensor_tensor(ln[:], kv[:, m, :], inv_kv[:], MUL)
                nc.sync.dma_start(agkv_in[128 * m : 128 * (m + 1), :], ln[:])
            # rope the decoupled key (shared across heads), feature-permuted on host
            cosk = a_res.tile([32, S], F32, name="cosk")
            sink = a_res.tile([32, S], F32, name="sink")
            nc.sync.dma_start(cosk[:], p_cosk[:, :])
            nc.sync.dma_start(sink[:], p_sink[:, :])
            kpe_r = a_wk.tile([64, S], F32, tag="kpe_r", name="kpe_r")
            _rope(nc, a_wk, kpe_r[:], kv[0:64, 4, :], cosk[:], sink[:], S, "ropek")
            nc.sync.dma_start(agkv_in[KVLORA : KVLORA + ROPE, :], kpe_r[:])
            # q_lat = wq_a' @ xh   -> [1536, S] feature-major
            ql = a_res.tile([128, QLORA // 128, S], F32, name="ql")
            for m in range(QLORA // 128):
                ps = a_ps.tile([128, S], F32, tag="a_mm", name="ql_ps")
                for k in range(D // 128):
                    wt = a_w.tile([128, 128], DT_AT, tag="wqa_t", name="wqa_t")
                    nc.sync.dma_start(
                        wt[:],
                        bc(p_wqa[128 * k : 128 * (k + 1), 128 * m : 128 * (m + 1)], DT_AT),
                    )
                    nc.tensor.matmul(
                        ps[:], wt[:], xh[:, k, :],
                        start=(k == 0), stop=(k == D // 128 - 1),
                    )
                nc.any.tensor_copy(ql[:, m, :], ps[:])
            inv_q = _rms_inv(
                nc, ones128, eps_sb[:], a_ps, a_wk,
                [ql[:, m, :] for m in range(QLORA // 128)], QLORA, S, "rmsq",
            )
            for m in range(QLORA // 128):
                qh = a_wk.tile([128, S], F32, tag="qh", name="qh")
                nc.vector.tensor_tensor(qh[:], ql[:, m, :], inv_q[:], MUL)
                nc.sync.dma_start(agq_in[128 * m : 128 * (m + 1), :], qh[:])


        nc.gpsimd.collective_compute(
            "AllGather", mybir.AluOpType.bypass, replica_groups=RG,
            ins=[agkv_in[:].opt()], outs=[agkv_out[:].opt()],
        )
        nc.gpsimd.collective_compute(
            "AllGather", mybir.AluOpType.bypass, replica_groups=RG,
            ins=[agq_in[:].opt()], outs=[agq_out[:].opt()],
        )

        # =========================== Stage B ===========================
        res1 = root.enter_context(tc.tile_pool(name="res1", bufs=1))
        res1_sb = res1.tile([128, DO // 128, N_TOK], F32, name="res1_sb")

        with ExitStack() as sb:
            b_res = sb.enter_context(tc.tile_pool(name="b_res", bufs=1))
            b_wk = sb.enter_context(tc.tile_pool(name="b_wk", bufs=2))

            k_sb = b_res.tile([128, HPC, N_TOK], DT_AT, name="k_sb")
            kpe_sb = b_res.tile([128, N_TOK], DT_AT, name="kpe_sb")
            v_sb = b_res.tile([128, N_TOK // 128, HPC * VHD], F32, name="v_sb")
            q_sb = b_res.tile([128, 3, N_TOK], DT_AT, name="q_sb")
            qpe_sb = b_res.tile([128, N_TOK], DT_AT, name="qpe_sb")
            y_sb = b_res.tile([128, HPC, N_TOK], F32, name="y_sb")

            # ---- expand k_nope and v (token-major) for the 2 local heads ----
            with ExitStack() as s1:
                b_w1 = s1.enter_context(tc.tile_pool(name="b_w1", bufs=1))
                b_rhs1 = s1.enter_context(tc.tile_pool(name="b_rhs1", bufs=8))
                b1_ps = s1.enter_context(
                    tc.tile_pool(name="b1_ps", bufs=2, space="PSUM")
                )
                wkb_sb = b_w1.tile([128, 4, HPC * NOPE], DT_V, name="wkb_sb")
                nc.sync.dma_start(
                    wkb_sb[:], bc(p_wkb.rearrange("(ko p) m -> p ko m", p=128), DT_V)
                )
                wv_sb = b_w1.tile([128, 4, HPC * VHD], DT_V, name="wv_sb")
                nc.sync.dma_start(
                    wv_sb[:], bc(p_wv.rearrange("(ko p) m -> p ko m", p=128), DT_V)
                )
                for blk in range(NCORES):
                    base = AGKV * blk
                    lat = [
                        b_rhs1.tile([128, S], DT_V, tag="lat", name="lat")
                        for _ in range(4)
                    ]
                    for k in range(4):
                        nc.sync.dma_start(
                            lat[k][:],
                            bc(agkv_out[base + 128 * k : base + 128 * (k + 1), :], DT_V),
                        )
                    for m in range(HPC):
                        psk = b1_ps.tile([128, S], F32, tag="psk", name="psk")
                        for k in range(4):
                            nc.tensor.matmul(
                                psk[:], wkb_sb[:, k, 128 * m : 128 * (m + 1)],
                                lat[k][:], start=(k == 0), stop=(k == 3),
                            )
                        nc.any.tensor_copy(k_sb[:, m, S * blk : S * (blk + 1)], psk[:])
                    for th in range(S // 128):
                        psv = b1_ps.tile([128, HPC * VHD], F32, tag="psv", name="psv")
                        for k in range(4):
                            nc.tensor.matmul(
                                psv[:], lat[k][:, 128 * th : 128 * (th + 1)],
                                wv_sb[:, k, :], start=(k == 0), stop=(k == 3),
                            )
                        nc.any.tensor_copy(v_sb[:, (S // 128) * blk + th, :], psv[:])
                    nc.sync.dma_start(
                        kpe_sb[0:64, S * blk : S * (blk + 1)],
                        bc(agkv_out[base + KVLORA : base + KVLORA + ROPE, :], DT_AT),
                    )
                    nc.sync.dma_start(
                        kpe_sb[64:128, S * blk : S * (blk + 1)],
                        bc(agkv_out[base + KVLORA : base + KVLORA + ROPE, :], DT_AT),
                    )

            # ---- q for the 2 local heads (fold SOFTSCALE here) ----
            with ExitStack() as s2:
                b_w2 = s2.enter_context(tc.tile_pool(name="b_w2", bufs=1))
                b_rhs2 = s2.enter_context(tc.tile_pool(name="b_rhs2", bufs=8))
                b2_ps = s2.enter_context(
                    tc.tile_pool(name="b2_ps", bufs=2, space="PSUM")
                )
                wqb_sb = b_w2.tile([128, QLORA // 128, HPC * QKD], DT_AT, name="wqb_sb")
                nc.sync.dma_start(
                    wqb_sb[:], bc(p_wqb.rearrange("(ko p) m -> p ko m", p=128), DT_AT)
                )
                for blk in range(NCORES):
                    base = AGQ * blk
                    qrhs = [
                        b_rhs2.tile([128, S], DT_AT, tag="qrhs", name="qrhs")
                        for _ in range(QLORA // 128)
                    ]
                    for k in range(QLORA // 128):
                        nc.sync.dma_start(
                            qrhs[k][:],
                            bc(agq_out[base + 128 * k : base + 128 * (k + 1), :], DT_AT),
                        )
                    psq = [
                        b2_ps.tile([128, S], F32, tag=f"psq{m}", name=f"psq{m}")
                        for m in range(3)
                    ]
                    for k in range(QLORA // 128):
                        for m in range(3):
                            nc.tensor.matmul(
                                psq[m][:], wqb_sb[:, k, 128 * m : 128 * (m + 1)],
                                qrhs[k][:],
                                start=(k == 0), stop=(k == QLORA // 128 - 1),
                            )
                    for m in range(3):
                        nc.scalar.mul(
                            q_sb[:, m, S * blk : S * (blk + 1)], psq[m][:], SOFTSCALE
                        )

            # rope q_pe: q_sb chunk 2 = [h0_e, h0_o, h1_e, h1_o] x32 rows
            with tc.tile_pool(name="rope_wk", bufs=1) as rp:
                for b in range(B):
                    for h in range(HPC):
                        _rope(
                            nc, rp,
                            qpe_sb[64 * h : 64 * (h + 1), T * b : T * (b + 1)],
                            q_sb[64 * h : 64 * (h + 1), 2, T * b : T * (b + 1)],
                            cosq[:, :], sinq[:, :], T, f"ropeq{b}{h}",
                        )

            # ---- attention (head-outer so y[h] finishes early for its AG) ----
            with ExitStack() as s3:
                b3_ps = s3.enter_context(
                    tc.tile_pool(name="b3_ps", bufs=2, space="PSUM")
                )
                for h in range(HPC):
                    for sc in range(T // 128):
                        W = 512 * (sc // 4 + 1)
                        mask_sb = b_wk.tile([128, T], F32, tag="mask_sb", name="mask_sb")
                        nc.sync.dma_start(
                            mask_sb[:, :W], p_mask[128 * sc : 128 * (sc + 1), :W]
                        )
                        for b in range(B):
                            s0 = T * b + 128 * sc
                            scs = b_wk.tile([128, T], F32, tag="scs", name="scs")
                            for tcx in range(W // 512):
                                t0 = T * b + 512 * tcx
                                ps = b3_ps.tile([128, 512], F32, tag="ps_qk", name="ps_qk")
                                nc.tensor.matmul(
                                    ps[:],
                                    q_sb[:, h, s0 : s0 + 128],
                                    k_sb[:, h, t0 : t0 + 512],
                                    start=True, stop=False,
                                )
                                nc.tensor.matmul(
                                    ps[:],
                                    qpe_sb[64 * h : 64 * (h + 1), s0 : s0 + 128],
                                    kpe_sb[64 * h : 64 * (h + 1), t0 : t0 + 512],
                                    start=False, stop=True,
                                )
                                nc.vector.tensor_tensor(
                                    scs[:, 512 * tcx : 512 * (tcx + 1)], ps[:],
                                    mask_sb[:, 512 * tcx : 512 * (tcx + 1)], ADD,
                                )
                            nmax = b_wk.tile([128, 1], F32, tag="nmax", name="nmax")
                            nc.vector.reduce_max(
                                nmax[:], scs[:, :W], axis=AX, negate=True
                            )
                            p_sb = b_wk.tile([128, T], F32, tag="p_sb", name="p_sb")
                            zsum = b_wk.tile([128, 1], F32, tag="zsum", name="zsum")
                            nc.scalar.activation(
                                p_sb[:, :W], scs[:, :W], AF.Exp,
                                bias=nmax[:], accum_out=zsum[:],
                            )
                            invz = b_wk.tile([128, 1], F32, tag="invz", name="invz")
                            nc.vector.reciprocal(invz[:], zsum[:])
                            nc.vector.tensor_scalar_mul(
                                p_sb[:, :W], p_sb[:, :W], invz[:]
                            )
                            ptr = b_wk.tile([128, T], F32, tag="ptr", name="ptr")
                            for tj in range(W // 128):
                                pst = b3_ps.tile([128, 128], F32, tag="pst", name="pst")
                                nc.tensor.transpose(
                                    pst[:], p_sb[:, 128 * tj : 128 * (tj + 1)], ident[:]
                                )
                                nc.any.tensor_copy(
                                    ptr[:, 128 * tj : 128 * (tj + 1)], pst[:]
                                )
                            psy = b3_ps.tile([128, 128], F32, tag="psy", name="psy")
                            ntj = W // 128
                            for tj in range(ntj):
                                nc.tensor.matmul(
                                    psy[:],
                                    v_sb[:, (T // 128) * b + tj, VHD * h : VHD * (h + 1)],
                                    ptr[:, 128 * tj : 128 * (tj + 1)],
                                    start=(tj == 0), stop=(tj == ntj - 1),
                                )
                            nc.any.tensor_copy(y_sb[:, h, s0 : s0 + 128], psy[:])

            for h in range(HPC):
                nc.sync.dma_start(y_in[h][:, :], y_sb[:, h, :])
                nc.gpsimd.collective_compute(
                    "AllGather", mybir.AluOpType.bypass, replica_groups=RG,
                    ins=[y_in[h][:].opt()], outs=[y_out[h][:].opt()],
                )

        # ================== Stage C: wo + norm + MLP ==================
        with ExitStack() as sc_stack:
            c_res = sc_stack.enter_context(tc.tile_pool(name="c_res", bufs=1))
            c_wk = sc_stack.enter_context(tc.tile_pool(name="c_wk", bufs=3))
            c_ps = sc_stack.enter_context(tc.tile_pool(name="c_ps", bufs=2, space="PSUM"))

            wo_sb = c_res.tile([128, H * VHD // 128, DO], DT_WO, name="wo_sb")
            nc.sync.dma_start(
                wo_sb[:], bc(p_wo.rearrange("(ko p) m -> p ko m", p=128), DT_WO)
            )
            for n in range(N_TOK // 512):
                pso = [
                    c_ps.tile([128, 512], F32, tag=f"pso{m}", name=f"pso{m}")
                    for m in range(DO // 128)
                ]
                for k in range(H * VHD // 128):
                    ysrc = y_out[0] if k < 8 else y_out[1]
                    kk = k % 8
                    yr = c_wk.tile([128, 512], DT_WO, tag="yr", name="yr")
                    nc.sync.dma_start(
                        yr[:],
                        bc(
                            ysrc[128 * kk : 128 * (kk + 1), 512 * n : 512 * (n + 1)],
                            DT_WO,
                        ),
                    )
                    for m in range(DO // 128):
                        nc.tensor.matmul(
                            pso[m][:], wo_sb[:, k, 128 * m : 128 * (m + 1)], yr[:],
                            start=(k == 0), stop=(k == H * VHD // 128 - 1),
                        )
                for m in range(DO // 128):
                    xt = c_wk.tile([128, 512], F32, tag="xt", name="xt")
                    nc.sync.dma_start(
                        xt[:], p_xf[128 * m : 128 * (m + 1), 512 * n : 512 * (n + 1)]
                    )
                    nc.vector.tensor_tensor(
                        res1_sb[:, m, 512 * n : 512 * (n + 1)], pso[m][:], xt[:], ADD
                    )

            # ffn-norm partial sum of squares (rows replicated via all-ones mm)
            ssp = c_res.tile([128, N_TOK], F32, name="ssp")
            for n in range(N_TOK // 512):
                pss = c_ps.tile([128, 512], F32, tag="pss", name="pss")
                for m in range(DO // 128):
                    xx = c_wk.tile([128, 512], F32, tag="ffn_xx", name="ffn_xx")
                    nc.scalar.square(xx[:], res1_sb[:, m, 512 * n : 512 * (n + 1)])
                    nc.tensor.matmul(
                        pss[:], ones128[:], xx[:],
                        start=(m == 0), stop=(m == DO // 128 - 1),
                    )
                nc.any.tensor_copy(ssp[:, 512 * n : 512 * (n + 1)], pss[:])
            nc.sync.dma_start(ar_in[:], ssp[:])
            nc.gpsimd.collective_compute(
                "AllReduce", mybir.AluOpType.add, replica_groups=RG,
                ins=[ar_in[:].opt()], outs=[ar_out[:].opt()],
            )
            ssf = c_res.tile([128, N_TOK], F32, name="ssf")
            nc.sync.dma_start(ssf[:], ar_out[:])
            stdf = c_res.tile([128, N_TOK], F32, name="stdf")
            nc.scalar.activation(stdf[:], ssf[:], AF.Sqrt, bias=eps_sb[:], scale=1.0 / D)
            invf = c_res.tile([128, N_TOK], F32, name="invf")
            nc.vector.reciprocal(invf[:], stdf[:])
            for half in range(2):
                for m in range(DO // 128):
                    h2t = c_wk.tile([128, T], F32, tag="h2t", name="h2t")
                    nc.vector.tensor_tensor(
                        h2t[:], res1_sb[:, m, T * half : T * (half + 1)],
                        invf[:, T * half : T * (half + 1)], MUL,
                    )
                    nc.sync.dma_start(h2_in[half][128 * m : 128 * (m + 1), :], h2t[:])
                nc.gpsimd.collective_compute(
                    "AllGather", mybir.AluOpType.bypass, replica_groups=RG,
                    ins=[h2_in[half][:].opt()], outs=[h2f[half][:].opt()],
                )

        # ---- MLP (TP over INTER), one pass per token half ----
        for half in range(2):
            with ExitStack() as sm:
                m_res = sm.enter_context(tc.tile_pool(name=f"m_res{half}", bufs=1))
                m_w = sm.enter_context(tc.tile_pool(name=f"m_w{half}", bufs=2))
                m_wk = sm.enter_context(tc.tile_pool(name=f"m_wk{half}", bufs=3))
                m_ps = sm.enter_context(
                    tc.tile_pool(name=f"m_ps{half}", bufs=2, space="PSUM")
                )
                hc = m_res.tile([128, D // 128, T], DT_MLP, name=f"hc{half}")
                nc.sync.dma_start(
                    hc[:],
                    bc(h2f[half][:].rearrange("(ko p) n -> p ko n", p=128), DT_MLP),
                )
                g_sb = m_res.tile([128, ISH // 128, T], DT_MLP, name=f"g{half}")
                for j in range(ISH // 128):
                    wj = m_w.tile([128, D // 128, 256], DT_MLP, tag="wj", name="wj")
                    nc.sync.dma_start(
                        wj[:],
                        bc(
                            p_w13[:, 256 * j : 256 * (j + 1)].rearrange(
                                "(ko p) m -> p ko m", p=128
                            ),
                            DT_MLP,
                        ),
                    )
                    for ns in range(T // 512):
                        psa = m_ps.tile([128, 512], F32, tag="psa", name="psa")
                        psb = m_ps.tile([128, 512], F32, tag="psb", name="psb")
                        for k in range(D // 128):
                            nc.tensor.matmul(
                                psa[:], wj[:, k, 0:128],
                                hc[:, k, 512 * ns : 512 * (ns + 1)],
                                start=(k == 0), stop=(k == D // 128 - 1),
                            )
                            nc.tensor.matmul(
                                psb[:], wj[:, k, 128:256],
                                hc[:, k, 512 * ns : 512 * (ns + 1)],
                                start=(k == 0), stop=(k == D // 128 - 1),
                            )
                        tsi = m_wk.tile([128, 512], F32, tag="tsi", name="tsi")
                        nc.scalar.activation(tsi[:], psa[:], AF.Silu)
                        nc.vector.tensor_tensor(
                            g_sb[:, j, 512 * ns : 512 * (ns + 1)], tsi[:], psb[:], MUL
                        )
                for m in range(D // 128):
                    w2m = m_w.tile([128, ISH // 128, 128], DT_MLP, tag="w2m", name="w2m")
                    nc.sync.dma_start(
                        w2m[:],
                        bc(
                            p_w2[:, 128 * m : 128 * (m + 1)].rearrange(
                                "(ko p) m2 -> p ko m2", p=128
                            ),
                            DT_MLP,
                        ),
                    )
                    for ns in range(T // 512):
                        ps2 = m_ps.tile([128, 512], F32, tag="ps2", name="ps2")
                        for k in range(ISH // 128):
                            nc.tensor.matmul(
                                ps2[:], w2m[:, k, :],
                                g_sb[:, k, 512 * ns : 512 * (ns + 1)],
                                start=(k == 0), stop=(k == ISH // 128 - 1),
                            )
                        o_sb = m_wk.tile([128, 512], F32, tag="o_sb", name="o_sb")
                        nc.any.tensor_copy(o_sb[:], ps2[:])
                        nc.sync.dma_start(
                            rs_in[half][
                                128 * m : 128 * (m + 1), 512 * ns : 512 * (ns + 1)
                            ],
                            o_sb[:],
                        )
            nc.gpsimd.collective_compute(
                "ReduceScatter", mybir.AluOpType.add, replica_groups=RG,
                ins=[rs_in[half][:].opt()], outs=[rs_out[half][:].opt()],
            )

        # ---- final residual add + output ----
        with ExitStack() as sf:
            f_wk = sf.enter_context(tc.tile_pool(name="f_wk", bufs=2))
            for half in range(2):
                for m in range(DO // 128):
                    rt = f_wk.tile([128, T], F32, tag="rt", name="rt")
                    nc.sync.dma_start(rt[:], rs_out[half][128 * m : 128 * (m + 1), :])
                    ot = f_wk.tile([128, T], F32, tag="ot", name="ot")
                    nc.vector.tensor_tensor(
                        ot[:], rt[:], res1_sb[:, m, T * half : T * (half + 1)], ADD
                    )
                    nc.sync.dma_start(
                        p_out[128 * m : 128 * (m + 1), T * half : T * (half + 1)], ot[:]
                    )

    nc.compile()
    return nc


def _rope_perm(n):
    """Permutation putting even lanes first then odd lanes, for an n-row rope
    block (n even): [0,2,4,...,n-2, 1,3,5,...,n-1]."""
    return np.concatenate([np.arange(0, n, 2), np.arange(1, n, 2)])


def kernel(**inputs):
    global _CACHED_NC, LAST_RESULTS, LAST_IN_MAPS
    f32 = lambda a: np.ascontiguousarray(np.asarray(a), dtype=np.float32)

    x = f32(inputs["x"]).reshape(N_TOK, D)
    mask = f32(inputs["mask"])
    cos = f32(inputs["freqs_cos"])  # [T, 32]
    sin = f32(inputs["freqs_sin"])
    attn_nw = f32(inputs["attn_norm_w"])
    wq_a = f32(inputs["wq_a"]) * attn_nw[None, :]
    q_nw = f32(inputs["q_norm_w"])
    wq_b = f32(inputs["wq_b"]) * q_nw[None, :]
    wkv_a = f32(inputs["wkv_a"]) * attn_nw[None, :]
    kv_nw = f32(inputs["kv_norm_w"])
    wkv_b = f32(inputs["wkv_b"]) * kv_nw[None, :]
    wo = f32(inputs["wo"])
    ffn_nw = f32(inputs["ffn_norm_w"])
    w1 = f32(inputs["w1"]) * ffn_nw[None, :]
    w3 = f32(inputs["w3"]) * ffn_nw[None, :]
    w2 = f32(inputs["w2"])

    xT = np.ascontiguousarray(x.T)  # [D, N_TOK] feature-major
    cosT = np.ascontiguousarray(cos.T)  # [32, T]
    sinT = np.ascontiguousarray(sin.T)

    # wkv_a rows: keep 0:512 (latent); permute rope rows 512:576 to even|odd
    pk = _rope_perm(ROPE)
    wkva_p = wkv_a.copy()
    wkva_p[KVLORA:] = wkv_a[KVLORA:][pk]
    wkvaT = np.ascontiguousarray(wkva_p.T)  # [D, 576]

    wqaT = np.ascontiguousarray(wq_a.T)  # [D, 1536]

    in_maps = []
    for c in range(NCORES):
        heads = [HPC * c + j for j in range(HPC)]
        # wq_b rows per head h: h*QKD .. h*QKD+192 (128 nope + 64 rope)
        # target col order: [h0_nope(128), h1_nope(128), h0_rope_eo(64), h1_rope_eo(64)]
        cols = []
        for h in heads:
            cols.append(wq_b[h * QKD : h * QKD + NOPE])
        for h in heads:
            cols.append(wq_b[h * QKD + NOPE : (h + 1) * QKD][pk])
        wqbT = np.ascontiguousarray(np.concatenate(cols, axis=0).T)  # [1536, 384]

        # wkv_b rows per head h: h*(NOPE+VHD) + [0:128]=k_nope, [128:256]=v
        kw = np.concatenate(
            [wkv_b[h * (NOPE + VHD) : h * (NOPE + VHD) + NOPE] for h in heads], axis=0
        )
        vw = np.concatenate(
            [wkv_b[h * (NOPE + VHD) + NOPE : (h + 1) * (NOPE + VHD)] for h in heads],
            axis=0,
        )
        wkbT = np.ascontiguousarray(kw.T)  # [512, 256]
        wvT = np.ascontiguousarray(vw.T)  # [512, 256]

        # wo rows (hv) reordered to [even heads | odd heads] to match the two
        # per-head y AllGathers (y_out[0] = heads 0,2,..,14; y_out[1] = odd)
        wo_sh = wo[DO * c : DO * (c + 1)]  # [256, 2048]
        hv_order = np.concatenate(
            [
                np.arange(h * VHD, (h + 1) * VHD)
                for h in list(range(0, H, 2)) + list(range(1, H, 2))
            ]
        )
        woT = np.ascontiguousarray(wo_sh[:, hv_order].T)  # [2048, 256]

        w1s = w1[ISH * c : ISH * (c + 1)]  # [1024, 2048]
        w3s = w3[ISH * c : ISH * (c + 1)]
        w13T = np.empty((D, 2 * ISH), np.float32)
        for j in range(ISH // 128):
            w13T[:, 256 * j : 256 * j + 128] = w1s[128 * j : 128 * (j + 1)].T
            w13T[:, 256 * j + 128 : 256 * (j + 1)] = w3s[128 * j : 128 * (j + 1)].T
        w2T = np.ascontiguousarray(w2[:, ISH * c : ISH * (c + 1)].T)  # [1024, 2048]

        tpos = (S * c) % T  # position within batch of this token shard
        in_maps.append(
            {
                "xs": np.ascontiguousarray(xT[:, S * c : S * (c + 1)]),
                "xf": np.ascontiguousarray(xT[DO * c : DO * (c + 1), :]),
                "mask": mask,
                "cosq": cosT,
                "sinq": sinT,
                "cosk": np.ascontiguousarray(cosT[:, tpos : tpos + S]),
                "sink": np.ascontiguousarray(sinT[:, tpos : tpos + S]),
                "wqaT": wqaT,
                "wqbT": wqbT,
                "wkvaT": wkvaT,
                "wkbT": wkbT,
                "wvT": wvT,
                "woT": woT,
                "w13T": np.ascontiguousarray(w13T),
                "w2T": w2T,
            }
        )

    LAST_IN_MAPS = in_maps
    if _CACHED_NC is None:
        _CACHED_NC = _build()
    nc = _CACHED_NC

    trace = bool(os.environ.get("KERNEL_TRACE"))
    res = run_bass_kernel_spmd(
        nc, in_maps, core_ids=list(range(NCORES)), trace=trace
    )
    LAST_RESULTS = res

    outT = np.concatenate([res.results[c]["out"] for c in range(NCORES)], axis=0)
    return np.ascontiguousarray(outT.T).reshape(B, T, D).astype(np.float32)
